# revision 47
# baseline (speedup 1.0000x reference)
"""Trainium2 Bass kernel for NeuralTensorDiagLayer (B=8192, D=K=2048).

Math: reference computes
    ff = concat([e1, e2], -1) @ V                      # (B, K)
    u  = ((e1*e2) @ W.T) / K                           # (B, K)
    z[p, q] = u[(p*D + q) % B, q]
    out = tanh(z + ff + b)

With D=2048, B=8192: (p*D + q) % B == 2048*(p % 4) + q, so z has only 4
distinct rows: zrow[r, q] = u[2048*r + q, q] — the diagonals of the four
2048x2048 blocks of u.  Hence the big u GEMM is unnecessary:
    zrow[r, q] = (1/K) * sum_j h[2048*r + q, j] * W[q, j],  h = e1*e2
which is an elementwise multiply + reduction only.

Distribution (8 cores, data-parallel over batch): core i owns rows
[1024*i, 1024*(i+1)); its zflat slice needs only its own e1/e2 shard and
a 1024-row slice of W, then a tiny AllGather replicates zflat.

Per-core device work (all GEMM operands bf16; fp32 PSUM keeps the error
~2e-3 << the 2e-2 gate):
  - main GEMM: out_T[q, p] = sum_k V[k, q] * XT[k, p] with XT =
    row-interleaved [e1s.T; e2s.T] resident in SBUF (V rows permuted
    identically on host), lhsT = 128-col slices of streamed V tiles.
  - z slice: t_j = e1t_j * e2t_j * (W_j / K) on DVE in bf16, summed
    across the 16 j-chunks into one [128, M] accumulator, then a single
    pair of ones-vector matmuls reduces the partitions.
  - epilogue: out_T = tanh(ff_T + zrow[p%4, q] + b[q]).

Bandwidth choreography (per-core HBM is ~300 GB/s shared by all
queues, so phase 1 cannot afford W): the sync ring carries the xt
batches FIRST, then W, then the last chunk's V columns, then V tiles
for groups C..G — ring serialization gives xt strict priority.  Group
0's V tiles and group B's ride the gpsimd ring in parallel.  The z
elementwise chain therefore runs after phase 1 (DVE is idle then), its
2-matmul reduction slots in after group C on the PE, and the AllGather
triggers at ~130us — far ahead of any consumer.

PSUM: chunks rotate round-robin over all 8 banks in groups of 2 (reuse
distance = 2 group windows, no bank-handoff stalls).  Groups A..D free
their banks via scalar-engine copies and defer the z-add / tanh / store
until zq has landed; groups E..H fuse the PSUM move and z-add into one
DVE tensor_add.  The final column chunk runs mb-major from a preloaded
V slice so only half a chunk's epilogue remains after the last matmul.
Host does sharding/layout only.
"""

import os
import ml_dtypes
import numpy as np

B, D, K = 8192, 2048, 2048
NCORES = 8
M = B // NCORES      # 1024 batch rows per core
KC = 2 * D           # 4096 contraction dim
P = 128
KT = KC // P         # 32 contraction chunks
NT = K // P          # 16 output-column chunks
JT = D // P          # 16 j-chunks for the z reduction
MBS = 512            # matmul moving free dim
NMB = M // MBS       # 2 m-blocks

# phase-1 group has 4 column chunks (8 matmuls per arriving k-chunk) so
# phase 1 is PE-limited even at full clock under worst-case HBM
# contention; it owns all 8 psum banks, freed by scalar copies
GROUP_A = [0, 1, 2, 3]
GROUPS2 = [[4, 5], [6, 7], [8, 9], [10, 11], [12, 13], [14]]
GTAGS = [[0, 1, 2, 3], [4, 5, 6, 7]]
LAST = 15                               # mb-major final chunk, tags 6,7
N_DEFER = 2  # GROUPS2 index below which epilogues are deferred (B,C)

_cache = {}
LAST_RESULT = None


def _build():
    import concourse.bass as bass
    import concourse.mybir as mybir
    import concourse.tile as tile
    from concourse import bacc
    from concourse.tile import add_dep_helper

    f32 = mybir.dt.float32
    bf16 = mybir.dt.bfloat16
    Act = mybir.ActivationFunctionType

    nc = bacc.Bacc(
        "TRN2", target_bir_lowering=False, debug=False, num_devices=NCORES
    )

    xt = nc.dram_tensor("xt", [KC, M], bf16, kind="ExternalInput").ap()
    v = nc.dram_tensor("v", [KC, K], bf16, kind="ExternalInput").ap()
    wt = nc.dram_tensor("wt", [D, M], bf16, kind="ExternalInput").ap()
    bvec = nc.dram_tensor("bvec", [K], f32, kind="ExternalInput").ap()
    out_t = nc.dram_tensor("out_t", [K, M], f32, kind="ExternalOutput").ap()

    with tile.TileContext(nc) as tc:
        with (
            tc.tile_pool(name="xtp", bufs=1) as xtp,
            tc.tile_pool(name="vt2p", bufs=6) as vt2p,
            tc.tile_pool(name="wtp", bufs=4) as wtp,
            tc.tile_pool(name="htp", bufs=1) as htp,
            tc.tile_pool(name="ttp", bufs=1) as ttp,
            tc.tile_pool(name="outp", bufs=8) as outp,
            tc.tile_pool(name="fusedp", bufs=2) as fusedp,
            tc.tile_pool(name="constp", bufs=1) as constp,
            tc.tile_pool(name="psg", bufs=1, space="PSUM") as psg,
            tc.tile_pool(name="dramp", bufs=1, space="DRAM") as dramp,
        ):
            ones = constp.tile([P, 1], bf16, name="ones", tag="ones")
            nc.vector.memset(ones[:], 1.0)
            # PE warmup: dummy matmuls from ~4us (right after the
            # memsets, long before the first data lands) so the HAM
            # clock-gate is at full rate when the real GEMM starts --
            # otherwise the first ~14 matmuls run at half clock.  The
            # stream must span until real data arrives (~13us): a >3.4us
            # idle gap would re-throttle.
            wrm = constp.tile([P, 256], bf16, name="wrm", tag="wrm")
            nc.vector.memset(wrm[:], 0.0)
            pwarm = psg.tile([1, 256], f32, name="pwarm", tag="ps6")
            for _ in range(48):
                nc.tensor.matmul(pwarm[:], ones[:], wrm[:],
                                 start=True, stop=True)
            b_all = constp.tile([P, NT], f32, name="b_all", tag="b_all")
            nc.scalar.dma_start(b_all[:], bvec.rearrange("(n q) -> q n", q=P))
            zq_all = constp.tile([P, NT, 4], f32, name="zq_all", tag="zq_all")

            # ---- phase-1 input streams: xt 2-chunk batches alternate
            # between the sync and gpsimd rings (one ring alone peaks
            # ~190 GB/s under whole-chip HBM contention — not enough for
            # a warm PE), with group-0 V tiles interleaved on gpsimd in
            # consumption order ----
            # DMA-ring arbitration shares bandwidth by outstanding
            # descriptors, so the xt stream gets the whole sync ring to
            # itself (16 queued descriptors win the arbiter), while the
            # gpsimd ring's group-A V tiles are dependency-chained one
            # xt batch behind — a trickle that cannot crowd xt out.
            AW = len(GROUP_A) * P          # group-A output width (cols)
            xtg = []
            xtg_dma = []
            vtsA = {}
            for j in range(JT):
                a, bnd = 2 * j, 2 * j + 2
                t = xtp.tile([P, 2 * M], bf16, name=f"xtg{j}",
                             tag=f"xtg{j}")
                xtg_dma.append(nc.sync.dma_start(
                    t[:].rearrange("p (c m) -> p c m", m=M),
                    xt[a * P : bnd * P, :].rearrange("(c p) m -> p c m",
                                                     p=P),
                ))
                xtg.append((a, bnd, t))
            for tt in range(8):
                vt = constp.tile([P, 4 * AW], bf16, name=f"vt3_{tt}",
                                 tag=f"vt3_{tt}")
                dma = nc.gpsimd.dma_start(
                    vt[:].rearrange("p (c q) -> p c q", q=AW),
                    v[4 * tt * P : (4 * tt + 4) * P, 0:AW].rearrange(
                        "(c p) q -> p c q", p=P),
                )
                add_dep_helper(
                    dma.ins, xtg_dma[max(2 * tt - 1, 0)].ins,
                    reason="vt3 trickles one xt batch behind",
                )
                for dk in range(4):
                    vtsA[4 * tt + dk] = vt[:, dk * AW : (dk + 1) * AW]

            def xts(kk):
                for a, bnd, t in xtg:
                    if a <= kk < bnd:
                        return t[:, (kk - a) * M : (kk - a + 1) * M]
                raise KeyError(kk)

            # ... then W, per-j-chunk tiles from a small rotating pool:
            # their issues are paced by the z-chain's consumption and sit
            # after all xt on the sync ring (the z path runs post-phase-1)
            wtl = []
            for j in range(JT):
                t = wtp.tile([P, M], bf16, name=f"wt{j}", tag="wtj")
                nc.gpsimd.dma_start(t[:], wt[j * P : (j + 1) * P, :])
                wtl.append(t)

            def wts(j):
                return wtl[j][:]

            # ... then the final chunk's V columns (used at the very end)
            vlast = constp.tile([P, KT * P], bf16, name="vlast", tag="vlast")
            nc.gpsimd.dma_start(
                vlast[:].rearrange("p (c q) -> p c q", q=P),
                v[:, LAST * P : (LAST + 1) * P].rearrange(
                    "(c p) q -> p c q", p=P),
            )

            def psum_tile(tag_idx):
                return psg.tile([P, MBS], f32, name=f"ps{tag_idx}",
                                tag=f"ps{tag_idx}")

            def chunk_matmuls(cols, pss, kk, vts):
                vt = vts[kk]
                for ci in range(len(cols)):
                    for mb in range(NMB):
                        nc.tensor.matmul(
                            pss[ci * NMB + mb][:],
                            vt[:, ci * P : (ci + 1) * P],
                            xts(kk)[:, mb * MBS : (mb + 1) * MBS],
                            start=(kk == 0), stop=(kk == KT - 1),
                        )

            def copy_chunk(n, pt0, pt1, deferred):
                """Scalar-engine PSUM->SBUF copy: frees the banks without
                touching the (busy) DVE; finish once zq_all lands."""
                osb = outp.tile([P, M], f32, name="osb")
                nc.scalar.activation(osb[:, 0:MBS], pt0[:], Act.Copy)
                nc.scalar.activation(osb[:, MBS:M], pt1[:], Act.Copy)
                deferred.append((n, osb))

            def zq_bcast(n):
                return zq_all[:, n : n + 1, :].broadcast_to(
                    [P, MBS // 4, 4])

            def finish_deferred(deferred, after_ins):
                """Deferred epilogues.  Their out-DMAs are forced to
                compile after the last input DMA (cross-ring dep): the
                DMA flow-control semaphores are a slot pool shared by
                all rings, and a zq-gated out-DMA sitting in an early
                slot stalls later weight-DMA issues (observed: 25us PE
                starvation at group E).  Pushed to the end, their
                completions gate nothing."""
                for n, osb in deferred:
                    zb = zq_bcast(n)
                    for mb in range(NMB):
                        h = osb[:, mb * MBS : (mb + 1) * MBS].rearrange(
                            "p (a r) -> p a r", r=4)
                        nc.vector.tensor_add(h, h, zb)
                    nc.scalar.activation(
                        osb[:], osb[:], Act.Tanh, bias=b_all[:, n : n + 1]
                    )
                    dma = nc.scalar.dma_start(out_t[n * P : (n + 1) * P, :],
                                              osb[:])
                    if after_ins is not None:
                        add_dep_helper(
                            dma.ins, after_ins.ins,
                            reason="deferred stores after last input DMA",
                        )
                deferred.clear()

            def finish_fused(n, pt0, pt1):
                """Fused epilogue: osb = psum + z in one DVE op per half
                (bank-freeing read), then tanh+bias, then store."""
                osb = fusedp.tile([P, M], f32, name="osbf")
                zb = zq_bcast(n)
                for mb, pt in ((0, pt0), (1, pt1)):
                    h = osb[:, mb * MBS : (mb + 1) * MBS].rearrange(
                        "p (a r) -> p a r", r=4)
                    nc.vector.tensor_add(h, pt[:].rearrange(
                        "p (a r) -> p a r", r=4), zb)
                nc.scalar.activation(
                    osb[:], osb[:], Act.Tanh, bias=b_all[:, n : n + 1]
                )
                nc.scalar.dma_start(out_t[n * P : (n + 1) * P, :], osb[:])

            # ---- phase 1: group-A GEMM paced by the xt stream ----
            pssA = [psum_tile(s) for s in range(8)]
            for kk in range(KT):
                chunk_matmuls(GROUP_A, pssA, kk, vtsA)

            # z elementwise chain on DVE (xt is fully resident and the
            # DVE idle by the time W lands)
            acc = constp.tile([P, M], bf16, name="acc", tag="acc")
            for j in range(JT):
                htj = htp.tile([P, M], bf16, name="htj")
                nc.vector.tensor_mul(htj[:], xts(2 * j), xts(2 * j + 1))
                if j == 0:
                    nc.vector.tensor_mul(acc[:], htj[:], wts(0))
                else:
                    ttj = ttp.tile([P, M], bf16, name="ttj")
                    nc.vector.tensor_mul(ttj[:], htj[:], wts(j))
                    nc.vector.tensor_add(acc[:], acc[:], ttj[:])

            deferred = []
            for ci, n in enumerate(GROUP_A):
                copy_chunk(n, pssA[ci * NMB], pssA[ci * NMB + 1], deferred)

            # ---- 2-chunk groups with rotating psum tags; V tiles are
            # 2-k-chunks wide to halve the DMA issue count ----
            last_vt_dma = [None]
            for gi, cols in enumerate(GROUPS2):
                ncol = len(cols)
                tags = GTAGS[gi % 2][: 2 * ncol]
                pss = [psum_tile(t) for t in tags]
                # gpsimd only for group B: everything emitted after the
                # z-block would sit behind the blocking collective there
                ring = nc.gpsimd if gi == 0 else nc.sync
                c0 = cols[0]
                for kk2 in range(0, KT, 2):
                    vt = vt2p.tile([P, 2, ncol * P], bf16, name="vt2",
                                   tag=f"vt2w{ncol}")
                    last_vt_dma[0] = ring.dma_start(
                        vt[:],
                        v[kk2 * P : (kk2 + 2) * P,
                          c0 * P : (c0 + ncol) * P].rearrange(
                            "(c p) q -> p c q", p=P),
                    )
                    for dk in range(2):
                        kk = kk2 + dk
                        for ci in range(ncol):
                            for mb in range(NMB):
                                nc.tensor.matmul(
                                    pss[ci * NMB + mb][:],
                                    vt[:, dk, ci * P : (ci + 1) * P],
                                    xts(kk)[:, mb * MBS : (mb + 1) * MBS],
                                    start=(kk == 0), stop=(kk == KT - 1),
                                )

                if gi < N_DEFER:
                    for ci, n in enumerate(cols):
                        copy_chunk(n, pss[ci * NMB], pss[ci * NMB + 1],
                                   deferred)
                else:
                    for ci, n in enumerate(cols):
                        finish_fused(n, pss[ci * NMB], pss[ci * NMB + 1])

                if gi == 0:
                    # z partition-reduction (2 PE instructions on banks
                    # 0/1, which group B just released) -> AllGather.
                    # Triggering here (vs later) hides the collective's
                    # ~60us skew+transport latency behind groups C..D.
                    pz0 = psg.tile([1, MBS], f32, name="pz0", tag="ps0")
                    pz1 = psg.tile([1, MBS], f32, name="pz1", tag="ps1")
                    nc.tensor.matmul(pz0[:], ones[:], acc[:, 0:MBS],
                                     start=True, stop=True)
                    nc.tensor.matmul(pz1[:], ones[:], acc[:, MBS:M],
                                     start=True, stop=True)
                    zsl = constp.tile([1, M], f32, name="zsl", tag="zsl")
                    nc.scalar.activation(zsl[:, 0:MBS], pz0[:], Act.Copy)
                    nc.scalar.activation(zsl[:, MBS:M], pz1[:], Act.Copy)
                    zin = dramp.tile([M], f32, name="zin", tag="zin")
                    zout = dramp.tile([B], f32, name="zout", tag="zout",
                                      addr_space="Shared")
                    nc.scalar.dma_start(zin[:], zsl[:])
                    nc.gpsimd.collective_compute(
                        "AllGather",
                        mybir.AluOpType.bypass,
                        replica_groups=[list(range(NCORES))],
                        ins=[zin[:].opt()],
                        outs=[zout[:].opt()],
                    )
                    # zq_all[qq, n, r] = zflat[2048*r + 128*n + qq]
                    for r in range(4):
                        nc.gpsimd.dma_start(
                            zq_all[:, :, r],
                            zout[r * D : (r + 1) * D].rearrange(
                                "(n q) -> q n", q=P),
                        )



            finish_deferred(deferred, last_vt_dma[0])

            # ---- final chunk, mb-major so the first half's epilogue
            # overlaps the second half's matmuls ----
            ppL = [psum_tile(6), psum_tile(7)]
            osbL = fusedp.tile([P, M], f32, name="osbL")
            zbL = zq_bcast(LAST)
            for mb in range(NMB):
                for kk in range(KT):
                    nc.tensor.matmul(
                        ppL[mb][:],
                        vlast[:, kk * P : (kk + 1) * P],
                        xts(kk)[:, mb * MBS : (mb + 1) * MBS],
                        start=(kk == 0), stop=(kk == KT - 1),
                    )
                # 256-col pieces so the very last TT/ACT/store pipeline
                for sub in range(2):
                    lo = mb * MBS + sub * (MBS // 2)
                    hi = lo + MBS // 2
                    h = osbL[:, lo:hi].rearrange("p (a r) -> p a r", r=4)
                    nc.vector.tensor_add(
                        h,
                        ppL[mb][:, sub * (MBS // 2) : (sub + 1) * (MBS // 2)]
                        .rearrange("p (a r) -> p a r", r=4),
                        zbL[:, : MBS // 8, :],
                    )
                    nc.scalar.activation(
                        osbL[:, lo:hi], osbL[:, lo:hi],
                        Act.Tanh, bias=b_all[:, LAST : LAST + 1],
                    )
                    nc.scalar.dma_start(
                        out_t[LAST * P : (LAST + 1) * P, lo:hi],
                        osbL[:, lo:hi],
                    )

    nc.compile()
    return nc


def _get_nc():
    nc = _cache.get("nc")
    if nc is None:
        nc = _build()
        _cache["nc"] = nc
    return nc


def _interleave_rows(top, bot):
    """[T; B] with 128-row blocks interleaved: T0,B0,T1,B1,..."""
    n = top.shape[0] // P
    return np.stack(
        [top.reshape(n, P, -1), bot.reshape(n, P, -1)], axis=1
    ).reshape(2 * n * P, -1)


def kernel(e1, e2, W, V, b):
    from concourse.bass_utils import run_bass_kernel_spmd

    nc = _get_nc()

    e1 = np.ascontiguousarray(np.asarray(e1, dtype=np.float32))
    e2 = np.ascontiguousarray(np.asarray(e2, dtype=np.float32))
    W = np.ascontiguousarray(np.asarray(W, dtype=np.float32))
    V = np.ascontiguousarray(np.asarray(V, dtype=np.float32))
    b = np.ascontiguousarray(np.asarray(b, dtype=np.float32))

    v16 = V.astype(ml_dtypes.bfloat16)
    v_p = np.ascontiguousarray(_interleave_rows(v16[:D], v16[D:]))
    in_maps = []
    for i in range(NCORES):
        sl = slice(i * M, (i + 1) * M)
        e1t = e1[sl].T.astype(ml_dtypes.bfloat16)
        e2t = e2[sl].T.astype(ml_dtypes.bfloat16)
        xt_i = np.ascontiguousarray(_interleave_rows(e1t, e2t))
        qlo = (i % 2) * M
        # 1/K scale folded into W (power of two — exact in fp32);
        # bf16 is plenty for the z term (|z| ~ 1e-3 vs |ff| ~ 1)
        wt_i = np.ascontiguousarray(
            (W[qlo : qlo + M].T * np.float32(1.0 / K)).astype(
                ml_dtypes.bfloat16
            )
        )
        in_maps.append({"xt": xt_i, "v": v_p, "wt": wt_i, "bvec": b})

    res = run_bass_kernel_spmd(nc, in_maps, list(range(NCORES)))
    global LAST_RESULT
    LAST_RESULT = res

    out = np.empty((B, K), dtype=np.float32)
    for i in range(NCORES):
        out[i * M : (i + 1) * M, :] = res.results[i]["out_t"].T
    return out


# revision 50
# speedup vs baseline: 1.2527x; 1.2527x over previous
"""Trainium2 Bass kernel for NeuralTensorDiagLayer (B=8192, D=K=2048).

Math: reference computes
    ff = concat([e1, e2], -1) @ V                      # (B, K)
    u  = ((e1*e2) @ W.T) / K                           # (B, K)
    z[p, q] = u[(p*D + q) % B, q]
    out = tanh(z + ff + b)

With D=2048, B=8192: (p*D + q) % B == 2048*(p % 4) + q, so z has only 4
distinct rows: zrow[r, q] = u[2048*r + q, q] — the diagonals of the four
2048x2048 blocks of u.  Hence the big u GEMM is unnecessary:
    zrow[r, q] = (1/K) * sum_j h[2048*r + q, j] * W[q, j],  h = e1*e2
which is an elementwise multiply + reduction only.

Distribution (8 cores, data-parallel over batch): core i owns rows
[1024*i, 1024*(i+1)); its zflat slice needs only its own e1/e2 shard and
a 1024-row slice of W, then a tiny AllGather replicates zflat.

Per-core device work (all GEMM operands bf16; fp32 PSUM keeps the error
~2e-3 << the 2e-2 gate):
  - main GEMM: out_T[q, p] = sum_k V[k, q] * XT[k, p] with XT =
    row-interleaved [e1s.T; e2s.T] resident in SBUF (V rows permuted
    identically on host), lhsT = 128-col slices of streamed V tiles.
  - z slice: t_j = e1t_j * e2t_j * (W_j / K) on DVE in bf16, summed
    across the 16 j-chunks into one [128, M] accumulator, then a single
    pair of ones-vector matmuls reduces the partitions.
  - epilogue: out_T = tanh(ff_T + zrow[p%4, q] + b[q]).

Bandwidth choreography (per-core HBM is ~300 GB/s shared by all
queues, so phase 1 cannot afford W): the sync ring carries the xt
batches FIRST, then W, then the last chunk's V columns, then V tiles
for groups C..G — ring serialization gives xt strict priority.  Group
0's V tiles and group B's ride the gpsimd ring in parallel.  The z
elementwise chain therefore runs after phase 1 (DVE is idle then), its
2-matmul reduction slots in after group C on the PE, and the AllGather
triggers at ~130us — far ahead of any consumer.

PSUM: chunks rotate round-robin over all 8 banks in groups of 2 (reuse
distance = 2 group windows, no bank-handoff stalls).  Groups A..D free
their banks via scalar-engine copies and defer the z-add / tanh / store
until zq has landed; groups E..H fuse the PSUM move and z-add into one
DVE tensor_add.  The final column chunk runs mb-major from a preloaded
V slice so only half a chunk's epilogue remains after the last matmul.
Host does sharding/layout only.
"""

import os
import ml_dtypes
import numpy as np

B, D, K = 8192, 2048, 2048
NCORES = 8
M = B // NCORES      # 1024 batch rows per core
KC = 2 * D           # 4096 contraction dim
P = 128
KT = KC // P         # 32 contraction chunks
NT = K // P          # 16 output-column chunks
JT = D // P          # 16 j-chunks for the z reduction
MBS = 512            # matmul moving free dim
NMB = M // MBS       # 2 m-blocks

# phase-1 group has 4 column chunks (8 matmuls per arriving k-chunk) so
# phase 1 is PE-limited even at full clock under worst-case HBM
# contention; it owns all 8 psum banks, freed by scalar copies
GROUP_A = [0, 1, 2, 3]
GROUPS2 = [[4, 5], [6, 7], [8, 9], [10, 11], [12, 13], [14]]
GTAGS = [[0, 1, 2, 3], [4, 5, 6, 7]]
LAST = 15                               # mb-major final chunk, tags 6,7
N_DEFER = 2  # GROUPS2 index below which epilogues are deferred (B,C)

_cache = {}
LAST_RESULT = None


def _build():
    import concourse.bass as bass
    import concourse.mybir as mybir
    import concourse.tile as tile
    from concourse import bacc
    from concourse.tile import add_dep_helper

    f32 = mybir.dt.float32
    bf16 = mybir.dt.bfloat16
    Act = mybir.ActivationFunctionType

    nc = bacc.Bacc(
        "TRN2", target_bir_lowering=False, debug=False, num_devices=NCORES
    )

    xt = nc.dram_tensor("xt", [KC, M], bf16, kind="ExternalInput").ap()
    v = nc.dram_tensor("v", [KC, K], bf16, kind="ExternalInput").ap()
    wt = nc.dram_tensor("wt", [D, M], bf16, kind="ExternalInput").ap()
    bvec = nc.dram_tensor("bvec", [K], f32, kind="ExternalInput").ap()
    out_t = nc.dram_tensor("out_t", [K, M], f32, kind="ExternalOutput").ap()

    with tile.TileContext(nc) as tc:
        with (
            tc.tile_pool(name="xtp", bufs=1) as xtp,
            tc.tile_pool(name="vt3p", bufs=8) as vt3p,
            tc.tile_pool(name="vt2p", bufs=6) as vt2p,
            tc.tile_pool(name="wtp", bufs=1) as wtp,
            tc.tile_pool(name="htp", bufs=1) as htp,
            tc.tile_pool(name="ttp", bufs=1) as ttp,
            tc.tile_pool(name="outp", bufs=8) as outp,
            tc.tile_pool(name="fusedp", bufs=2) as fusedp,
            tc.tile_pool(name="constp", bufs=1) as constp,
            tc.tile_pool(name="psg", bufs=1, space="PSUM") as psg,
            tc.tile_pool(name="dramp", bufs=1, space="DRAM") as dramp,
        ):
            ones = constp.tile([P, 1], bf16, name="ones", tag="ones")
            nc.vector.memset(ones[:], 1.0)
            # PE warmup: dummy matmuls from ~4us (right after the
            # memsets, long before the first data lands) so the HAM
            # clock-gate is at full rate when the real GEMM starts --
            # otherwise the first ~14 matmuls run at half clock.  The
            # stream must span until real data arrives (~13us): a >3.4us
            # idle gap would re-throttle.
            wrm = constp.tile([P, 256], bf16, name="wrm", tag="wrm")
            nc.vector.memset(wrm[:], 0.0)
            pwarm = psg.tile([1, 256], f32, name="pwarm", tag="ps6")
            for _ in range(48):
                nc.tensor.matmul(pwarm[:], ones[:], wrm[:],
                                 start=True, stop=True)
            b_all = constp.tile([P, NT], f32, name="b_all", tag="b_all")
            nc.scalar.dma_start(b_all[:], bvec.rearrange("(n q) -> q n", q=P))
            zq_all = constp.tile([P, NT, 4], f32, name="zq_all", tag="zq_all")

            # ---- phase-1 input streams: xt 2-chunk batches alternate
            # between the sync and gpsimd rings (one ring alone peaks
            # ~190 GB/s under whole-chip HBM contention — not enough for
            # a warm PE), with group-0 V tiles interleaved on gpsimd in
            # consumption order ----
            # The xt stream owns the sync ring (16 queued descriptors win
            # the per-descriptor ring arbitration); group-A V tiles ride
            # the gpsimd ring from a small pool whose buffer reuse gates
            # naturally pace them at PE consumption rate.
            AW = len(GROUP_A) * P          # group-A output width (cols)
            xtg = []
            vtsA = {}
            for j in range(JT):
                a, bnd = 2 * j, 2 * j + 2
                t = xtp.tile([P, 2 * M], bf16, name=f"xtg{j}",
                             tag=f"xtg{j}")
                nc.sync.dma_start(
                    t[:].rearrange("p (c m) -> p c m", m=M),
                    xt[a * P : bnd * P, :].rearrange("(c p) m -> p c m",
                                                     p=P),
                )
                xtg.append((a, bnd, t))
            for kk in range(KT):
                vt = vt3p.tile([P, AW], bf16, name="vt3", tag="vt3")
                nc.gpsimd.dma_start(vt[:], v[kk * P : (kk + 1) * P, 0:AW])
                vtsA[kk] = vt[:]

            def xts(kk):
                for a, bnd, t in xtg:
                    if a <= kk < bnd:
                        return t[:, (kk - a) * M : (kk - a + 1) * M]
                raise KeyError(kk)

            # ... then W on the sync ring behind all of xt (the z path
            # only consumes it after phase 1)
            wtg = []
            for bi in range(4):
                a, bnd = 4 * bi, 4 * bi + 4
                t = wtp.tile([P, 4 * M], bf16, name=f"wtg{bi}",
                             tag=f"wtg{bi}")
                nc.sync.dma_start(
                    t[:].rearrange("p (c m) -> p c m", m=M),
                    wt[a * P : bnd * P, :].rearrange("(c p) m -> p c m",
                                                     p=P),
                )
                wtg.append(t)

            def wts(j):
                return wtg[j // 4][:, (j % 4) * M : (j % 4 + 1) * M]

            # ... then the final chunk's V columns (used at the very end)
            vlast = constp.tile([P, KT * P], bf16, name="vlast", tag="vlast")
            nc.gpsimd.dma_start(
                vlast[:].rearrange("p (c q) -> p c q", q=P),
                v[:, LAST * P : (LAST + 1) * P].rearrange(
                    "(c p) q -> p c q", p=P),
            )

            def psum_tile(tag_idx):
                return psg.tile([P, MBS], f32, name=f"ps{tag_idx}",
                                tag=f"ps{tag_idx}")

            def chunk_matmuls(cols, pss, kk, vts):
                vt = vts[kk]
                for ci in range(len(cols)):
                    for mb in range(NMB):
                        nc.tensor.matmul(
                            pss[ci * NMB + mb][:],
                            vt[:, ci * P : (ci + 1) * P],
                            xts(kk)[:, mb * MBS : (mb + 1) * MBS],
                            start=(kk == 0), stop=(kk == KT - 1),
                        )

            def copy_chunk(n, pt0, pt1, deferred):
                """Scalar-engine PSUM->SBUF copy: frees the banks without
                touching the (busy) DVE; finish once zq_all lands."""
                osb = outp.tile([P, M], f32, name="osb")
                nc.scalar.activation(osb[:, 0:MBS], pt0[:], Act.Copy)
                nc.scalar.activation(osb[:, MBS:M], pt1[:], Act.Copy)
                deferred.append((n, osb))

            def zq_bcast(n):
                return zq_all[:, n : n + 1, :].broadcast_to(
                    [P, MBS // 4, 4])

            def finish_deferred(deferred, after_ins):
                """Deferred epilogues.  Their out-DMAs are forced to
                compile after the last input DMA (cross-ring dep): the
                DMA flow-control semaphores are a slot pool shared by
                all rings, and a zq-gated out-DMA sitting in an early
                slot stalls later weight-DMA issues (observed: 25us PE
                starvation at group E).  Pushed to the end, their
                completions gate nothing."""
                for n, osb in deferred:
                    zb = zq_bcast(n)
                    for mb in range(NMB):
                        h = osb[:, mb * MBS : (mb + 1) * MBS].rearrange(
                            "p (a r) -> p a r", r=4)
                        nc.vector.tensor_add(h, h, zb)
                    nc.scalar.activation(
                        osb[:], osb[:], Act.Tanh, bias=b_all[:, n : n + 1]
                    )
                    dma = nc.scalar.dma_start(out_t[n * P : (n + 1) * P, :],
                                              osb[:])
                    if after_ins is not None:
                        add_dep_helper(
                            dma.ins, after_ins.ins,
                            reason="deferred stores after last input DMA",
                        )
                deferred.clear()

            def finish_fused(n, pt0, pt1):
                """Fused epilogue: osb = psum + z in one DVE op per half
                (bank-freeing read), then tanh+bias, then store."""
                osb = fusedp.tile([P, M], f32, name="osbf")
                zb = zq_bcast(n)
                for mb, pt in ((0, pt0), (1, pt1)):
                    h = osb[:, mb * MBS : (mb + 1) * MBS].rearrange(
                        "p (a r) -> p a r", r=4)
                    nc.vector.tensor_add(h, pt[:].rearrange(
                        "p (a r) -> p a r", r=4), zb)
                nc.scalar.activation(
                    osb[:], osb[:], Act.Tanh, bias=b_all[:, n : n + 1]
                )
                nc.scalar.dma_start(out_t[n * P : (n + 1) * P, :], osb[:])

            # ---- phase 1: group-A GEMM paced by the xt stream ----
            pssA = [psum_tile(s) for s in range(8)]
            for kk in range(KT):
                chunk_matmuls(GROUP_A, pssA, kk, vtsA)

            # z elementwise chain on DVE (xt is fully resident and the
            # DVE idle by the time W lands)
            acc = constp.tile([P, M], bf16, name="acc", tag="acc")
            for j in range(JT):
                htj = htp.tile([P, M], bf16, name="htj")
                nc.vector.tensor_mul(htj[:], xts(2 * j), xts(2 * j + 1))
                if j == 0:
                    nc.vector.tensor_mul(acc[:], htj[:], wts(0))
                else:
                    ttj = ttp.tile([P, M], bf16, name="ttj")
                    nc.vector.tensor_mul(ttj[:], htj[:], wts(j))
                    nc.vector.tensor_add(acc[:], acc[:], ttj[:])

            deferred = []
            for ci, n in enumerate(GROUP_A):
                copy_chunk(n, pssA[ci * NMB], pssA[ci * NMB + 1], deferred)

            # ---- 2-chunk groups with rotating psum tags; V tiles are
            # 2-k-chunks wide to halve the DMA issue count ----
            last_vt_dma = [None]
            for gi, cols in enumerate(GROUPS2):
                ncol = len(cols)
                tags = GTAGS[gi % 2][: 2 * ncol]
                pss = [psum_tile(t) for t in tags]
                # gpsimd only for group B: everything emitted after the
                # z-block would sit behind the blocking collective there
                ring = nc.gpsimd if gi == 0 else nc.sync
                c0 = cols[0]
                for kk2 in range(0, KT, 2):
                    vt = vt2p.tile([P, 2, ncol * P], bf16, name="vt2",
                                   tag=f"vt2w{ncol}")
                    last_vt_dma[0] = ring.dma_start(
                        vt[:],
                        v[kk2 * P : (kk2 + 2) * P,
                          c0 * P : (c0 + ncol) * P].rearrange(
                            "(c p) q -> p c q", p=P),
                    )
                    for dk in range(2):
                        kk = kk2 + dk
                        for ci in range(ncol):
                            for mb in range(NMB):
                                nc.tensor.matmul(
                                    pss[ci * NMB + mb][:],
                                    vt[:, dk, ci * P : (ci + 1) * P],
                                    xts(kk)[:, mb * MBS : (mb + 1) * MBS],
                                    start=(kk == 0), stop=(kk == KT - 1),
                                )

                if gi < N_DEFER:
                    for ci, n in enumerate(cols):
                        copy_chunk(n, pss[ci * NMB], pss[ci * NMB + 1],
                                   deferred)
                else:
                    for ci, n in enumerate(cols):
                        finish_fused(n, pss[ci * NMB], pss[ci * NMB + 1])

                if gi == 0:
                    # z partition-reduction (2 PE instructions on banks
                    # 0/1, which group B just released) -> AllGather.
                    # Triggering here (vs later) hides the collective's
                    # ~60us skew+transport latency behind groups C..D.
                    pz0 = psg.tile([1, MBS], f32, name="pz0", tag="ps0")
                    pz1 = psg.tile([1, MBS], f32, name="pz1", tag="ps1")
                    nc.tensor.matmul(pz0[:], ones[:], acc[:, 0:MBS],
                                     start=True, stop=True)
                    nc.tensor.matmul(pz1[:], ones[:], acc[:, MBS:M],
                                     start=True, stop=True)
                    zsl = constp.tile([1, M], f32, name="zsl", tag="zsl")
                    nc.scalar.activation(zsl[:, 0:MBS], pz0[:], Act.Copy)
                    nc.scalar.activation(zsl[:, MBS:M], pz1[:], Act.Copy)
                    zin = dramp.tile([M], f32, name="zin", tag="zin")
                    zout = dramp.tile([B], f32, name="zout", tag="zout",
                                      addr_space="Shared")
                    nc.scalar.dma_start(zin[:], zsl[:])
                    nc.gpsimd.collective_compute(
                        "AllGather",
                        mybir.AluOpType.bypass,
                        replica_groups=[list(range(NCORES))],
                        ins=[zin[:].opt()],
                        outs=[zout[:].opt()],
                    )
                    # zq_all[qq, n, r] = zflat[2048*r + 128*n + qq]
                    for r in range(4):
                        nc.gpsimd.dma_start(
                            zq_all[:, :, r],
                            zout[r * D : (r + 1) * D].rearrange(
                                "(n q) -> q n", q=P),
                        )



            finish_deferred(deferred, last_vt_dma[0])

            # ---- final chunk, mb-major so the first half's epilogue
            # overlaps the second half's matmuls ----
            ppL = [psum_tile(6), psum_tile(7)]
            osbL = fusedp.tile([P, M], f32, name="osbL")
            zbL = zq_bcast(LAST)
            for mb in range(NMB):
                for kk in range(KT):
                    nc.tensor.matmul(
                        ppL[mb][:],
                        vlast[:, kk * P : (kk + 1) * P],
                        xts(kk)[:, mb * MBS : (mb + 1) * MBS],
                        start=(kk == 0), stop=(kk == KT - 1),
                    )
                # 256-col pieces so the very last TT/ACT/store pipeline
                for sub in range(2):
                    lo = mb * MBS + sub * (MBS // 2)
                    hi = lo + MBS // 2
                    h = osbL[:, lo:hi].rearrange("p (a r) -> p a r", r=4)
                    nc.vector.tensor_add(
                        h,
                        ppL[mb][:, sub * (MBS // 2) : (sub + 1) * (MBS // 2)]
                        .rearrange("p (a r) -> p a r", r=4),
                        zbL[:, : MBS // 8, :],
                    )
                    nc.scalar.activation(
                        osbL[:, lo:hi], osbL[:, lo:hi],
                        Act.Tanh, bias=b_all[:, LAST : LAST + 1],
                    )
                    nc.scalar.dma_start(
                        out_t[LAST * P : (LAST + 1) * P, lo:hi],
                        osbL[:, lo:hi],
                    )

    nc.compile()
    return nc


def _get_nc():
    nc = _cache.get("nc")
    if nc is None:
        nc = _build()
        _cache["nc"] = nc
    return nc


def _interleave_rows(top, bot):
    """[T; B] with 128-row blocks interleaved: T0,B0,T1,B1,..."""
    n = top.shape[0] // P
    return np.stack(
        [top.reshape(n, P, -1), bot.reshape(n, P, -1)], axis=1
    ).reshape(2 * n * P, -1)


def kernel(e1, e2, W, V, b):
    from concourse.bass_utils import run_bass_kernel_spmd

    nc = _get_nc()

    e1 = np.ascontiguousarray(np.asarray(e1, dtype=np.float32))
    e2 = np.ascontiguousarray(np.asarray(e2, dtype=np.float32))
    W = np.ascontiguousarray(np.asarray(W, dtype=np.float32))
    V = np.ascontiguousarray(np.asarray(V, dtype=np.float32))
    b = np.ascontiguousarray(np.asarray(b, dtype=np.float32))

    v16 = V.astype(ml_dtypes.bfloat16)
    v_p = np.ascontiguousarray(_interleave_rows(v16[:D], v16[D:]))
    in_maps = []
    for i in range(NCORES):
        sl = slice(i * M, (i + 1) * M)
        e1t = e1[sl].T.astype(ml_dtypes.bfloat16)
        e2t = e2[sl].T.astype(ml_dtypes.bfloat16)
        xt_i = np.ascontiguousarray(_interleave_rows(e1t, e2t))
        qlo = (i % 2) * M
        # 1/K scale folded into W (power of two — exact in fp32);
        # bf16 is plenty for the z term (|z| ~ 1e-3 vs |ff| ~ 1)
        wt_i = np.ascontiguousarray(
            (W[qlo : qlo + M].T * np.float32(1.0 / K)).astype(
                ml_dtypes.bfloat16
            )
        )
        in_maps.append({"xt": xt_i, "v": v_p, "wt": wt_i, "bvec": b})

    res = run_bass_kernel_spmd(nc, in_maps, list(range(NCORES)))
    global LAST_RESULT
    LAST_RESULT = res

    out = np.empty((B, K), dtype=np.float32)
    for i in range(NCORES):
        out[i * M : (i + 1) * M, :] = res.results[i]["out_t"].T
    return out


# revision 51
# speedup vs baseline: 1.2571x; 1.0035x over previous
"""Trainium2 Bass kernel for NeuralTensorDiagLayer (B=8192, D=K=2048).

Math: reference computes
    ff = concat([e1, e2], -1) @ V                      # (B, K)
    u  = ((e1*e2) @ W.T) / K                           # (B, K)
    z[p, q] = u[(p*D + q) % B, q]
    out = tanh(z + ff + b)

With D=2048, B=8192: (p*D + q) % B == 2048*(p % 4) + q, so z has only 4
distinct rows: zrow[r, q] = u[2048*r + q, q] — the diagonals of the four
2048x2048 blocks of u.  Hence the big u GEMM is unnecessary:
    zrow[r, q] = (1/K) * sum_j h[2048*r + q, j] * W[q, j],  h = e1*e2
which is an elementwise multiply + reduction only.

Distribution (8 cores, data-parallel over batch): core i owns rows
[1024*i, 1024*(i+1)); its zflat slice needs only its own e1/e2 shard and
a 1024-row slice of W, then a tiny AllGather replicates zflat.

Per-core device work (all GEMM operands bf16; fp32 PSUM keeps the error
~2e-3 << the 2e-2 gate):
  - main GEMM: out_T[q, p] = sum_k V[k, q] * XT[k, p] with XT =
    row-interleaved [e1s.T; e2s.T] resident in SBUF (V rows permuted
    identically on host), lhsT = 128-col slices of streamed V tiles.
  - z slice: t_j = e1t_j * e2t_j * (W_j / K) on DVE in bf16, summed
    across the 16 j-chunks into one [128, M] accumulator, then a single
    pair of ones-vector matmuls reduces the partitions.
  - epilogue: out_T = tanh(ff_T + zrow[p%4, q] + b[q]).

Bandwidth choreography (per-core HBM is ~300 GB/s shared by all
queues, so phase 1 cannot afford W): the sync ring carries the xt
batches FIRST, then W, then the last chunk's V columns, then V tiles
for groups C..G — ring serialization gives xt strict priority.  Group
0's V tiles and group B's ride the gpsimd ring in parallel.  The z
elementwise chain therefore runs after phase 1 (DVE is idle then), its
2-matmul reduction slots in after group C on the PE, and the AllGather
triggers at ~130us — far ahead of any consumer.

PSUM: chunks rotate round-robin over all 8 banks in groups of 2 (reuse
distance = 2 group windows, no bank-handoff stalls).  Groups A..D free
their banks via scalar-engine copies and defer the z-add / tanh / store
until zq has landed; groups E..H fuse the PSUM move and z-add into one
DVE tensor_add.  The final column chunk runs mb-major from a preloaded
V slice so only half a chunk's epilogue remains after the last matmul.
Host does sharding/layout only.
"""

import os
import ml_dtypes
import numpy as np

B, D, K = 8192, 2048, 2048
NCORES = 8
M = B // NCORES      # 1024 batch rows per core
KC = 2 * D           # 4096 contraction dim
P = 128
KT = KC // P         # 32 contraction chunks
NT = K // P          # 16 output-column chunks
JT = D // P          # 16 j-chunks for the z reduction
MBS = 512            # matmul moving free dim
NMB = M // MBS       # 2 m-blocks

# phase-1 group has 4 column chunks (8 matmuls per arriving k-chunk) so
# phase 1 is PE-limited even at full clock under worst-case HBM
# contention; it owns all 8 psum banks, freed by scalar copies
GROUP_A = [0, 1, 2, 3]
GROUPS2 = [[4, 5], [6, 7], [8, 9], [10, 11], [12, 13], [14]]
GTAGS = [[0, 1, 2, 3], [4, 5, 6, 7]]
LAST = 15                               # mb-major final chunk, tags 6,7
N_DEFER = 2  # GROUPS2 index below which epilogues are deferred (B,C)

_cache = {}
LAST_RESULT = None


def _build():
    import concourse.bass as bass
    import concourse.mybir as mybir
    import concourse.tile as tile
    from concourse import bacc
    from concourse.tile import add_dep_helper

    f32 = mybir.dt.float32
    bf16 = mybir.dt.bfloat16
    Act = mybir.ActivationFunctionType

    nc = bacc.Bacc(
        "TRN2", target_bir_lowering=False, debug=False, num_devices=NCORES
    )

    xt = nc.dram_tensor("xt", [KC, M], bf16, kind="ExternalInput").ap()
    v = nc.dram_tensor("v", [KC, K], bf16, kind="ExternalInput").ap()
    wt = nc.dram_tensor("wt", [D, M], bf16, kind="ExternalInput").ap()
    bvec = nc.dram_tensor("bvec", [K], f32, kind="ExternalInput").ap()
    out_t = nc.dram_tensor("out_t", [K, M], f32, kind="ExternalOutput").ap()

    with tile.TileContext(nc) as tc:
        with (
            tc.tile_pool(name="xtp", bufs=1) as xtp,
            tc.tile_pool(name="vt3p", bufs=8) as vt3p,
            tc.tile_pool(name="vt2p", bufs=6) as vt2p,
            tc.tile_pool(name="wtp", bufs=1) as wtp,
            tc.tile_pool(name="htp", bufs=1) as htp,
            tc.tile_pool(name="ttp", bufs=1) as ttp,
            tc.tile_pool(name="outp", bufs=8) as outp,
            tc.tile_pool(name="fusedp", bufs=2) as fusedp,
            tc.tile_pool(name="constp", bufs=1) as constp,
            tc.tile_pool(name="psg", bufs=1, space="PSUM") as psg,
            tc.tile_pool(name="dramp", bufs=1, space="DRAM") as dramp,
        ):
            ones = constp.tile([P, 1], bf16, name="ones", tag="ones")
            nc.vector.memset(ones[:], 1.0)
            # PE warmup: dummy matmuls from ~4us (right after the
            # memsets, long before the first data lands) so the HAM
            # clock-gate is at full rate when the real GEMM starts --
            # otherwise the first ~14 matmuls run at half clock.  The
            # stream must span until real data arrives (~13us): a >3.4us
            # idle gap would re-throttle.
            wrm = constp.tile([P, 256], bf16, name="wrm", tag="wrm")
            nc.vector.memset(wrm[:], 0.0)
            pwarm = psg.tile([1, 256], f32, name="pwarm", tag="ps6")
            for _ in range(48):
                nc.tensor.matmul(pwarm[:], ones[:], wrm[:],
                                 start=True, stop=True)
            b_all = constp.tile([P, NT], f32, name="b_all", tag="b_all")
            nc.scalar.dma_start(b_all[:], bvec.rearrange("(n q) -> q n", q=P))
            zq_all = constp.tile([P, NT, 4], f32, name="zq_all", tag="zq_all")

            # ---- phase-1 input streams: xt 2-chunk batches alternate
            # between the sync and gpsimd rings (one ring alone peaks
            # ~190 GB/s under whole-chip HBM contention — not enough for
            # a warm PE), with group-0 V tiles interleaved on gpsimd in
            # consumption order ----
            # The xt stream owns the sync ring (16 queued descriptors win
            # the per-descriptor ring arbitration); group-A V tiles ride
            # the gpsimd ring from a small pool whose buffer reuse gates
            # naturally pace them at PE consumption rate.
            AW = len(GROUP_A) * P          # group-A output width (cols)
            xtg = []
            vtsA = {}
            bounds = [0, 1, 2] + [2 * j for j in range(2, JT + 1)]
            for bi in range(len(bounds) - 1):
                a, bnd = bounds[bi], bounds[bi + 1]
                t = xtp.tile([P, (bnd - a) * M], bf16, name=f"xtg{bi}",
                             tag=f"xtg{bi}")
                nc.sync.dma_start(
                    t[:].rearrange("p (c m) -> p c m", m=M),
                    xt[a * P : bnd * P, :].rearrange("(c p) m -> p c m",
                                                     p=P),
                )
                xtg.append((a, bnd, t))
            for kk in range(KT):
                vt = vt3p.tile([P, AW], bf16, name="vt3", tag="vt3")
                nc.gpsimd.dma_start(vt[:], v[kk * P : (kk + 1) * P, 0:AW])
                vtsA[kk] = vt[:]

            def xts(kk):
                for a, bnd, t in xtg:
                    if a <= kk < bnd:
                        return t[:, (kk - a) * M : (kk - a + 1) * M]
                raise KeyError(kk)

            # ... then W on the sync ring behind all of xt (the z path
            # only consumes it after phase 1)
            wtg = []
            for bi in range(4):
                a, bnd = 4 * bi, 4 * bi + 4
                t = wtp.tile([P, 4 * M], bf16, name=f"wtg{bi}",
                             tag=f"wtg{bi}")
                nc.sync.dma_start(
                    t[:].rearrange("p (c m) -> p c m", m=M),
                    wt[a * P : bnd * P, :].rearrange("(c p) m -> p c m",
                                                     p=P),
                )
                wtg.append(t)

            def wts(j):
                return wtg[j // 4][:, (j % 4) * M : (j % 4 + 1) * M]

            # ... then the final chunk's V columns (used at the very end)
            vlast = constp.tile([P, KT * P], bf16, name="vlast", tag="vlast")
            nc.gpsimd.dma_start(
                vlast[:].rearrange("p (c q) -> p c q", q=P),
                v[:, LAST * P : (LAST + 1) * P].rearrange(
                    "(c p) q -> p c q", p=P),
            )

            def psum_tile(tag_idx):
                return psg.tile([P, MBS], f32, name=f"ps{tag_idx}",
                                tag=f"ps{tag_idx}")

            def chunk_matmuls(cols, pss, kk, vts):
                vt = vts[kk]
                for ci in range(len(cols)):
                    for mb in range(NMB):
                        nc.tensor.matmul(
                            pss[ci * NMB + mb][:],
                            vt[:, ci * P : (ci + 1) * P],
                            xts(kk)[:, mb * MBS : (mb + 1) * MBS],
                            start=(kk == 0), stop=(kk == KT - 1),
                        )

            def copy_chunk(n, pt0, pt1, deferred):
                """Scalar-engine PSUM->SBUF copy: frees the banks without
                touching the (busy) DVE; finish once zq_all lands."""
                osb = outp.tile([P, M], f32, name="osb")
                nc.scalar.activation(osb[:, 0:MBS], pt0[:], Act.Copy)
                nc.scalar.activation(osb[:, MBS:M], pt1[:], Act.Copy)
                deferred.append((n, osb))

            def zq_bcast(n):
                return zq_all[:, n : n + 1, :].broadcast_to(
                    [P, MBS // 4, 4])

            def finish_deferred(deferred, after_ins):
                """Deferred epilogues.  Their out-DMAs are forced to
                compile after the last input DMA (cross-ring dep): the
                DMA flow-control semaphores are a slot pool shared by
                all rings, and a zq-gated out-DMA sitting in an early
                slot stalls later weight-DMA issues (observed: 25us PE
                starvation at group E).  Pushed to the end, their
                completions gate nothing."""
                for n, osb in deferred:
                    zb = zq_bcast(n)
                    for mb in range(NMB):
                        h = osb[:, mb * MBS : (mb + 1) * MBS].rearrange(
                            "p (a r) -> p a r", r=4)
                        nc.vector.tensor_add(h, h, zb)
                    nc.scalar.activation(
                        osb[:], osb[:], Act.Tanh, bias=b_all[:, n : n + 1]
                    )
                    dma = nc.scalar.dma_start(out_t[n * P : (n + 1) * P, :],
                                              osb[:])
                    if after_ins is not None:
                        add_dep_helper(
                            dma.ins, after_ins.ins,
                            reason="deferred stores after last input DMA",
                        )
                deferred.clear()

            def finish_fused(n, pt0, pt1):
                """Fused epilogue: osb = psum + z in one DVE op per half
                (bank-freeing read), then tanh+bias, then store."""
                osb = fusedp.tile([P, M], f32, name="osbf")
                zb = zq_bcast(n)
                for mb, pt in ((0, pt0), (1, pt1)):
                    h = osb[:, mb * MBS : (mb + 1) * MBS].rearrange(
                        "p (a r) -> p a r", r=4)
                    nc.vector.tensor_add(h, pt[:].rearrange(
                        "p (a r) -> p a r", r=4), zb)
                nc.scalar.activation(
                    osb[:], osb[:], Act.Tanh, bias=b_all[:, n : n + 1]
                )
                nc.scalar.dma_start(out_t[n * P : (n + 1) * P, :], osb[:])

            # ---- phase 1: group-A GEMM paced by the xt stream ----
            pssA = [psum_tile(s) for s in range(8)]
            for kk in range(KT):
                chunk_matmuls(GROUP_A, pssA, kk, vtsA)

            # z elementwise chain on DVE (xt is fully resident and the
            # DVE idle by the time W lands)
            acc = constp.tile([P, M], bf16, name="acc", tag="acc")
            for j in range(JT):
                htj = htp.tile([P, M], bf16, name="htj")
                nc.vector.tensor_mul(htj[:], xts(2 * j), xts(2 * j + 1))
                if j == 0:
                    nc.vector.tensor_mul(acc[:], htj[:], wts(0))
                else:
                    ttj = ttp.tile([P, M], bf16, name="ttj")
                    nc.vector.tensor_mul(ttj[:], htj[:], wts(j))
                    nc.vector.tensor_add(acc[:], acc[:], ttj[:])

            deferred = []
            for ci, n in enumerate(GROUP_A):
                copy_chunk(n, pssA[ci * NMB], pssA[ci * NMB + 1], deferred)

            # ---- 2-chunk groups with rotating psum tags; V tiles are
            # 2-k-chunks wide to halve the DMA issue count ----
            last_vt_dma = [None]
            for gi, cols in enumerate(GROUPS2):
                ncol = len(cols)
                tags = GTAGS[gi % 2][: 2 * ncol]
                pss = [psum_tile(t) for t in tags]
                # gpsimd only for group B: everything emitted after the
                # z-block would sit behind the blocking collective there
                ring = nc.gpsimd if gi == 0 else nc.sync
                c0 = cols[0]
                for kk2 in range(0, KT, 2):
                    vt = vt2p.tile([P, 2, ncol * P], bf16, name="vt2",
                                   tag=f"vt2w{ncol}")
                    last_vt_dma[0] = ring.dma_start(
                        vt[:],
                        v[kk2 * P : (kk2 + 2) * P,
                          c0 * P : (c0 + ncol) * P].rearrange(
                            "(c p) q -> p c q", p=P),
                    )
                    for dk in range(2):
                        kk = kk2 + dk
                        for ci in range(ncol):
                            for mb in range(NMB):
                                nc.tensor.matmul(
                                    pss[ci * NMB + mb][:],
                                    vt[:, dk, ci * P : (ci + 1) * P],
                                    xts(kk)[:, mb * MBS : (mb + 1) * MBS],
                                    start=(kk == 0), stop=(kk == KT - 1),
                                )

                if gi < N_DEFER:
                    for ci, n in enumerate(cols):
                        copy_chunk(n, pss[ci * NMB], pss[ci * NMB + 1],
                                   deferred)
                else:
                    for ci, n in enumerate(cols):
                        finish_fused(n, pss[ci * NMB], pss[ci * NMB + 1])

                if gi == 0:
                    # z partition-reduction (2 PE instructions on banks
                    # 0/1, which group B just released) -> AllGather.
                    # Triggering here (vs later) hides the collective's
                    # ~60us skew+transport latency behind groups C..D.
                    pz0 = psg.tile([1, MBS], f32, name="pz0", tag="ps0")
                    pz1 = psg.tile([1, MBS], f32, name="pz1", tag="ps1")
                    nc.tensor.matmul(pz0[:], ones[:], acc[:, 0:MBS],
                                     start=True, stop=True)
                    nc.tensor.matmul(pz1[:], ones[:], acc[:, MBS:M],
                                     start=True, stop=True)
                    zsl = constp.tile([1, M], f32, name="zsl", tag="zsl")
                    nc.scalar.activation(zsl[:, 0:MBS], pz0[:], Act.Copy)
                    nc.scalar.activation(zsl[:, MBS:M], pz1[:], Act.Copy)
                    zin = dramp.tile([M], f32, name="zin", tag="zin")
                    zout = dramp.tile([B], f32, name="zout", tag="zout",
                                      addr_space="Shared")
                    nc.scalar.dma_start(zin[:], zsl[:])
                    nc.gpsimd.collective_compute(
                        "AllGather",
                        mybir.AluOpType.bypass,
                        replica_groups=[list(range(NCORES))],
                        ins=[zin[:].opt()],
                        outs=[zout[:].opt()],
                    )
                    # zq_all[qq, n, r] = zflat[2048*r + 128*n + qq]
                    for r in range(4):
                        nc.gpsimd.dma_start(
                            zq_all[:, :, r],
                            zout[r * D : (r + 1) * D].rearrange(
                                "(n q) -> q n", q=P),
                        )



            finish_deferred(deferred, last_vt_dma[0])

            # ---- final chunk, mb-major so the first half's epilogue
            # overlaps the second half's matmuls ----
            ppL = [psum_tile(6), psum_tile(7)]
            osbL = fusedp.tile([P, M], f32, name="osbL")
            zbL = zq_bcast(LAST)
            for mb in range(NMB):
                for kk in range(KT):
                    nc.tensor.matmul(
                        ppL[mb][:],
                        vlast[:, kk * P : (kk + 1) * P],
                        xts(kk)[:, mb * MBS : (mb + 1) * MBS],
                        start=(kk == 0), stop=(kk == KT - 1),
                    )
                # 256-col pieces so the very last TT/ACT/store pipeline
                for sub in range(2):
                    lo = mb * MBS + sub * (MBS // 2)
                    hi = lo + MBS // 2
                    h = osbL[:, lo:hi].rearrange("p (a r) -> p a r", r=4)
                    nc.vector.tensor_add(
                        h,
                        ppL[mb][:, sub * (MBS // 2) : (sub + 1) * (MBS // 2)]
                        .rearrange("p (a r) -> p a r", r=4),
                        zbL[:, : MBS // 8, :],
                    )
                    nc.scalar.activation(
                        osbL[:, lo:hi], osbL[:, lo:hi],
                        Act.Tanh, bias=b_all[:, LAST : LAST + 1],
                    )
                    nc.scalar.dma_start(
                        out_t[LAST * P : (LAST + 1) * P, lo:hi],
                        osbL[:, lo:hi],
                    )

    nc.compile()
    return nc


def _get_nc():
    nc = _cache.get("nc")
    if nc is None:
        nc = _build()
        _cache["nc"] = nc
    return nc


def _interleave_rows(top, bot):
    """[T; B] with 128-row blocks interleaved: T0,B0,T1,B1,..."""
    n = top.shape[0] // P
    return np.stack(
        [top.reshape(n, P, -1), bot.reshape(n, P, -1)], axis=1
    ).reshape(2 * n * P, -1)


def kernel(e1, e2, W, V, b):
    from concourse.bass_utils import run_bass_kernel_spmd

    nc = _get_nc()

    e1 = np.ascontiguousarray(np.asarray(e1, dtype=np.float32))
    e2 = np.ascontiguousarray(np.asarray(e2, dtype=np.float32))
    W = np.ascontiguousarray(np.asarray(W, dtype=np.float32))
    V = np.ascontiguousarray(np.asarray(V, dtype=np.float32))
    b = np.ascontiguousarray(np.asarray(b, dtype=np.float32))

    v16 = V.astype(ml_dtypes.bfloat16)
    v_p = np.ascontiguousarray(_interleave_rows(v16[:D], v16[D:]))
    in_maps = []
    for i in range(NCORES):
        sl = slice(i * M, (i + 1) * M)
        e1t = e1[sl].T.astype(ml_dtypes.bfloat16)
        e2t = e2[sl].T.astype(ml_dtypes.bfloat16)
        xt_i = np.ascontiguousarray(_interleave_rows(e1t, e2t))
        qlo = (i % 2) * M
        # 1/K scale folded into W (power of two — exact in fp32);
        # bf16 is plenty for the z term (|z| ~ 1e-3 vs |ff| ~ 1)
        wt_i = np.ascontiguousarray(
            (W[qlo : qlo + M].T * np.float32(1.0 / K)).astype(
                ml_dtypes.bfloat16
            )
        )
        in_maps.append({"xt": xt_i, "v": v_p, "wt": wt_i, "bvec": b})

    res = run_bass_kernel_spmd(nc, in_maps, list(range(NCORES)))
    global LAST_RESULT
    LAST_RESULT = res

    out = np.empty((B, K), dtype=np.float32)
    for i in range(NCORES):
        out[i * M : (i + 1) * M, :] = res.results[i]["out_t"].T
    return out
